# revision 7
# baseline (speedup 1.0000x reference)
"""Pointer-attention kernel for Trainium2 (8 NeuronCores, data-parallel over batch).

Computes, for P = pointer_input [B, S, R], weights W1/W2 [2R]:
    scores = P @ W1[:R] + (h @ W1[R:])[:, None]      # h-term is constant over S
    a      = softmax(scores, axis=S)                 #   -> cancels in softmax
    c      = einsum('bsr,bs->br', P, a)
    pi     = P @ W2[:R] + (c @ W2[R:])[:, None]

Math used here (exact):
    s1[b,s]  = P[b,s,:] . w1p          (w1p = W1[:R])
    E        = exp(s1)                 (softmax shift cancels; inputs are O(1))
    Z[b]     = sum_s E[b,s]
    craw[b,:]= sum_s E[b,s] * P[b,s,:]
    g[b]     = (craw[b,:] . w2c) / Z[b]            (w2c = W2[R:])
    pi[b,s]  = P[b,s,:] . w2p + g[b]               (w2p = W2[:R])

so h_t and W1[R:] never affect the output. One single pass over P.

Per core: 8 batches. Per (b, s-tile of 128): DMA [128,512]; two DVE
scalar_tensor_tensor matvecs (s1, pw2); ScalarE exp; one TensorE matmul
accumulating craw (lhsT = E column, rhs = P tile). Per-b epilogue does the
tiny reductions and the broadcast add.
"""

import numpy as np

B, S, R = 64, 2048, 512
N_CORES = 8
B_LOC = B // N_CORES          # 8 batches per core
P_PART = 128                  # partitions per s-tile
NT = S // P_PART              # 16 s-tiles per batch

_CACHED_NC = None


def _build_nc(b_loc=B_LOC, nt=NT, finalize=True):
    import concourse.bacc as bacc
    import concourse.bass as bass
    import concourse.mybir as mybir
    import concourse.tile as tile

    f32 = mybir.dt.float32
    s_loc = nt * P_PART
    nc = bacc.Bacc(None, target_bir_lowering=False, debug=True)

    p_h = nc.declare_dram_parameter("p", [b_loc, s_loc, R], f32, isOutput=False)
    w1_h = nc.declare_dram_parameter("w1", [2 * R], f32, isOutput=False)
    w2_h = nc.declare_dram_parameter("w2", [2 * R], f32, isOutput=False)
    out_h = nc.declare_dram_parameter("out", [b_loc, s_loc], f32, isOutput=True)

    def bcast_ap(src_ap, parts):
        # replicate a 1-D DRAM slice across `parts` partitions
        return bass.AP(
            tensor=src_ap.tensor,
            offset=src_ap.offset,
            ap=[[0, parts]] + [list(d) for d in src_ap.ap],
        )

    with tile.TileContext(nc) as tc:
        with (
            tc.tile_pool(name="consts", bufs=1) as consts,
            tc.tile_pool(name="ptiles", bufs=6) as ptiles,
            tc.tile_pool(name="scratch", bufs=4) as scratch,
            tc.tile_pool(name="perb", bufs=3) as perb,
            tc.tile_pool(name="smalls", bufs=3) as smalls,
            tc.tile_pool(name="psum_c", bufs=2, space="PSUM") as psum_c,
            tc.tile_pool(name="psum_s", bufs=2, space="PSUM") as psum_s,
        ):
            # ---- constants ----
            w1p = consts.tile([P_PART, R], f32)
            nc.gpsimd.dma_start(out=w1p[:], in_=bcast_ap(w1_h[0:R], P_PART))
            w2p = consts.tile([P_PART, R], f32)
            nc.gpsimd.dma_start(out=w2p[:], in_=bcast_ap(w2_h[0:R], P_PART))
            w2c = consts.tile([1, R], f32)
            nc.gpsimd.dma_start(out=w2c[:], in_=bcast_ap(w2_h[R : 2 * R], 1))
            ones_col = consts.tile([P_PART, 1], f32)
            nc.vector.memset(ones_col[:], 1.0)
            ones_row = consts.tile([1, P_PART], f32)
            nc.vector.memset(ones_row[:], 1.0)

            for b in range(b_loc):
                c_ps = psum_c.tile([1, R], f32, tag="c_ps")
                s1_b = perb.tile([P_PART, nt], f32, tag="s1_b")
                pw2_b = perb.tile([P_PART, nt], f32, tag="pw2_b")
                e_b = perb.tile([P_PART, nt], f32, tag="e_b")

                for t in range(nt):
                    pt = ptiles.tile([P_PART, R], f32, tag="pt")
                    nc.sync.dma_start(
                        out=pt[:], in_=p_h[b, t * P_PART : (t + 1) * P_PART, :]
                    )
                    prod1 = scratch.tile([P_PART, R], f32, tag="prod1")
                    nc.vector.scalar_tensor_tensor(
                        out=prod1[:],
                        in0=pt[:],
                        scalar=1.0,
                        in1=w1p[:],
                        op0=mybir.AluOpType.mult,
                        op1=mybir.AluOpType.mult,
                        accum_out=s1_b[:, t : t + 1],
                    )
                    prod2 = scratch.tile([P_PART, R], f32, tag="prod2")
                    nc.vector.scalar_tensor_tensor(
                        out=prod2[:],
                        in0=pt[:],
                        scalar=1.0,
                        in1=w2p[:],
                        op0=mybir.AluOpType.mult,
                        op1=mybir.AluOpType.mult,
                        accum_out=pw2_b[:, t : t + 1],
                    )
                    nc.scalar.activation(
                        out=e_b[:, t : t + 1],
                        in_=s1_b[:, t : t + 1],
                        func=mybir.ActivationFunctionType.Exp,
                    )
                    nc.tensor.matmul(
                        c_ps[:],
                        lhsT=e_b[:, t : t + 1],
                        rhs=pt[:],
                        start=(t == 0),
                        stop=(t == nt - 1),
                    )

                # ---- per-batch epilogue ----
                es = smalls.tile([P_PART, 1], f32, tag="es")
                nc.vector.reduce_sum(es[:], e_b[:], axis=mybir.AxisListType.X)
                z_ps = psum_s.tile([1, 1], f32, tag="z_ps")
                nc.tensor.matmul(
                    z_ps[:], lhsT=es[:], rhs=ones_col[:], start=True, stop=True
                )
                c_sb = smalls.tile([1, R], f32, tag="c_sb")
                nc.scalar.copy(out=c_sb[:], in_=c_ps[:])
                zr = smalls.tile([1, 1], f32, tag="zr")
                nc.vector.reciprocal(out=zr[:], in_=z_ps[:])
                cprod = smalls.tile([1, R], f32, tag="cprod")
                dq = smalls.tile([1, 1], f32, tag="dq")
                nc.vector.scalar_tensor_tensor(
                    out=cprod[:],
                    in0=c_sb[:],
                    scalar=1.0,
                    in1=w2c[:],
                    op0=mybir.AluOpType.mult,
                    op1=mybir.AluOpType.mult,
                    accum_out=dq[:],
                )
                g = smalls.tile([1, 1], f32, tag="g")
                nc.vector.tensor_mul(g[:], dq[:], zr[:])
                g_ps = psum_s.tile([P_PART, 1], f32, tag="g_ps")
                nc.tensor.matmul(
                    g_ps[:], lhsT=ones_row[:], rhs=g[:], start=True, stop=True
                )
                g_bc = smalls.tile([P_PART, 1], f32, tag="g_bc")
                nc.scalar.copy(out=g_bc[:], in_=g_ps[:])
                pi_b = perb.tile([P_PART, nt], f32, tag="pi_b")
                nc.scalar.activation(
                    out=pi_b[:],
                    in_=pw2_b[:],
                    func=mybir.ActivationFunctionType.Identity,
                    bias=g_bc[:],
                    scale=1.0,
                )
                nc.sync.dma_start(
                    out=out_h[b].rearrange("(t p) -> p t", p=P_PART),
                    in_=pi_b[:],
                )

    if finalize:
        nc.finalize()
    return nc


def _get_nc():
    global _CACHED_NC
    if _CACHED_NC is None:
        _CACHED_NC = _build_nc()
    return _CACHED_NC


def run_sharded(pointer_input, W1, W2, trace=False, trace_kwargs=None):
    """Run the SPMD kernel; returns (full_output [1,B,S], BassKernelResults)."""
    from concourse.bass_utils import run_bass_kernel_spmd

    nc = _get_nc()
    pointer_input = np.ascontiguousarray(pointer_input, dtype=np.float32)
    W1 = np.ascontiguousarray(W1, dtype=np.float32)
    W2 = np.ascontiguousarray(W2, dtype=np.float32)
    in_maps = [
        {
            "p": pointer_input[i * B_LOC : (i + 1) * B_LOC],
            "w1": W1,
            "w2": W2,
        }
        for i in range(N_CORES)
    ]
    kw = dict(trace_kwargs or {})
    res = run_bass_kernel_spmd(
        nc, in_maps, list(range(N_CORES)), trace=trace, **kw
    )
    out = np.concatenate([res.results[i]["out"] for i in range(N_CORES)], axis=0)
    return out[None].astype(np.float32), res


def kernel(pointer_input, h_t, W1, W2):
    # h_t only shifts scores by a per-batch constant, which softmax cancels;
    # it does not affect the output.
    out, _ = run_sharded(pointer_input, W1, W2, trace=False)
    return out
